# revision 13
# baseline (speedup 1.0000x reference)
"""GCN 2-layer forward on 8 Trainium2 NeuronCores (Bass/Tile).

Pull-model, dst-sharded: per layer, out = D^-1/2 (A+I) D^-1/2 (X W) + b.
The weight matmul commutes with the aggregation, so each core:
  1. gathers rows of the dinv-prescaled fp16 feature table (x' = dinv*x)
     for its edges via SWDGE dma_gather (128B reads, 256B-strided table),
  2. segment-sums them with one-hot matmuls on the PE
     (level 1: 128-edge chunk x 32-slot one-hot -> partials, PSUM f32;
      level 2: 128-partial tile x 128-dst window one-hot -> AGG, PSUM f32),
  3. adds the self-loop term with a PE transpose-matmul (rhs = I),
  4. applies W (fp16), bias, dinv scaling and activation densely,
  5. allgathers layer-1 activations (h1' = dinv*relu(...)) for layer 2.
One-hot matrices are built on the DVE by comparing host-provided segment ids
to an iota table (fp16 is exact for these small ints).  Edges are grouped by
(src-bucket, dst-window) with fixed chunk counts so the single SPMD program
is valid for every core; int16 gather indices address equal-size buckets.
"""
import numpy as np

import concourse.bass as bass
import concourse.bacc as bacc
import concourse.mybir as mybir
import concourse.tile as tile
import concourse.ap_utils as ap_utils
from concourse.alu_op_type import AluOpType
from concourse import bass_utils

AFT = mybir.ActivationFunctionType
F16 = mybir.dt.float16
F32 = mybir.dt.float32
I16 = mybir.dt.int16

N_CORES = 8
WIN = 128          # dst rows per level-2 window
NSLOT = 32         # level-1 one-hot width
CPT = 4            # chunks per partial tile (32*4 = 128 slots)
WG = 16            # windows per gather call / L1 batch
L2B = 4            # windows per L2 batch


class Cfg:
    def __init__(self, n, e, f_in=64, hid=64, ncls=40):
        assert n % N_CORES == 0
        self.N, self.E, self.F, self.HID, self.NCLS = n, e, f_in, hid, ncls
        self.SHARD = n // N_CORES
        self.NWIN = (self.SHARD + WIN - 1) // WIN
        self.SHARD_PAD = self.NWIN * WIN
        self.NPAD = self.SHARD_PAD * N_CORES
        self.NBUCK = -(-self.NPAD // 32768)
        self.BUCKSZ = -(-self.NPAD // self.NBUCK)
        self.WGROUPS = [(w0, min(WG, self.NWIN - w0))
                        for w0 in range(0, self.NWIN, WG)]
        self.L2GROUPS = [(w0, min(L2B, self.NWIN - w0))
                         for w0 in range(0, self.NWIN, L2B)]

    def derive(self, ncwb):
        """Structure derived from per-bucket chunk counts."""
        self.ncwb = list(ncwb)                       # chunks per (w, b)
        self.npt = [-(-c // CPT) for c in self.ncwb]  # partial tiles per (w, b)
        self.tbase = np.concatenate([[0], np.cumsum(self.npt)]).tolist()
        self.TPW = int(self.tbase[-1])               # partial tiles per window
        self.segoff = np.concatenate(
            [[0], np.cumsum([self.NWIN * c for c in self.ncwb])]).tolist()
        self.idxoff = []                             # srcidx col offset per call
        off = 0
        for b in range(self.NBUCK):
            for (w0, wc) in self.WGROUPS:
                self.idxoff.append(off)
                off += wc * self.ncwb[b] * WIN // 16
        self.IDXCOLS = off


def _slot_of(cfg, g):
    c = g // cfg.SHARD
    r = g % cfg.SHARD
    return c * cfg.SHARD_PAD + (r % WIN) * cfg.NWIN + (r // WIN)


def host_prep(cfg, x, edge_index, W1, b1, W2, b2):
    src = np.asarray(edge_index[0]).astype(np.int64)
    dst = np.asarray(edge_index[1]).astype(np.int64)
    deg = np.bincount(dst, minlength=cfg.N).astype(np.float64) + 1.0
    dinv = (1.0 / np.sqrt(deg)).astype(np.float32)

    xp = (dinv[:, None] * np.asarray(x, np.float32)).astype(np.float16)
    slot_all = _slot_of(cfg, np.arange(cfg.N, dtype=np.int64))
    xfull = np.zeros((cfg.NPAD, 2 * cfg.F), np.float16)
    xfull[slot_all, : cfg.F] = xp

    eslot = slot_all[src]
    ecore = dst // cfg.SHARD
    ebuck = eslot // cfg.BUCKSZ
    edloc = dst % cfg.SHARD

    # chunk cutting per (core, window, bucket): <=WIN edges and <=NSLOT
    # distinct dsts per chunk (greedy).  Produces per-edge (chunk, pos, seg).
    def cut_groups(eb, ew, dl):
        """edges pre-sorted by (b, w, dl); returns k, p, segloc arrays and
        per-(b,) max chunk count."""
        n = len(dl)
        k = np.zeros(n, np.int64)
        p = np.zeros(n, np.int64)
        segloc = np.zeros(n, np.int64)
        ncwb = np.zeros(cfg.NBUCK, np.int64)
        key = eb * cfg.NWIN + ew
        cnt = np.bincount(key, minlength=cfg.NBUCK * cfg.NWIN)
        start = np.zeros(len(cnt) + 1, np.int64)
        np.cumsum(cnt, out=start[1:])
        for g in range(cfg.NBUCK * cfg.NWIN):
            a, bnd = start[g], start[g + 1]
            if bnd == a:
                continue
            dls = dl[a:bnd]
            newd = np.empty(bnd - a, bool)
            newd[0] = True
            np.not_equal(dls[1:], dls[:-1], out=newd[1:])
            dr = np.cumsum(newd) - 1
            i, ck = 0, 0
            m_ = bnd - a
            while i < m_:
                jmax = min(m_, i + WIN)
                if dr[jmax - 1] - dr[i] + 1 <= NSLOT:
                    j = jmax
                else:
                    j = i + int(np.searchsorted(dr[i:jmax], dr[i] + NSLOT))
                k[a + i:a + j] = ck
                p[a + i:a + j] = np.arange(j - i)
                segloc[a + i:a + j] = dr[i:j] - dr[i]
                i = j
                ck += 1
            b = g // cfg.NWIN
            ncwb[b] = max(ncwb[b], ck)
        return k, p, segloc, ncwb

    # derive global per-bucket chunk counts from all cores
    ncwb_g = np.zeros(cfg.NBUCK, np.int64)
    percore = []
    for c in range(N_CORES):
        m = ecore == c
        es, eb, ew = eslot[m], ebuck[m], edloc[m] // WIN
        dl = (edloc[m] % WIN).astype(np.int64)
        order = np.lexsort((dl, ew, eb))
        es, eb, ew, dl = es[order], eb[order], ew[order], dl[order]
        k, p, segloc, ncwb_c = cut_groups(eb, ew, dl)
        ncwb_g = np.maximum(ncwb_g, ncwb_c)
        percore.append((es, eb, ew, dl, k, p, segloc))
    cfg.derive([max(1, int(v)) for v in ncwb_g])

    iota32 = np.tile(np.arange(NSLOT, dtype=np.float16)[None, :], (128, 1))
    iota128 = np.tile(np.arange(WIN, dtype=np.float16)[None, :], (128, 1))
    consts = {
        "iota32": iota32, "iota128": iota128,
        "eye128": np.eye(WIN, dtype=np.float16),
        "eyeh": np.eye(cfg.HID, dtype=np.float16),
        "eyec": np.eye(cfg.NCLS, dtype=np.float16),
        "w1": np.asarray(W1, np.float32).astype(np.float16),
        "w2": np.asarray(W2, np.float32).astype(np.float16),
        "b1t": np.tile(np.asarray(b1, np.float32)[None, :], (128, 1)),
        "b2t": np.tile(np.asarray(b2, np.float32)[None, :], (128, 1)),
    }

    in_maps = []
    for c in range(N_CORES):
        es, eb, ew, dl, k, p, segloc = percore[c]

        # srcidx: per-call wrapped int16 columns
        idx_cols = np.zeros((16, cfg.IDXCOLS), np.int16)
        # seg: columns (b, w, k)
        seg_u = np.full((128, cfg.segoff[-1]), -1.0, np.float16)
        # dstrel: rows 32*(k%CPT)+s, columns w*TPW + tbase[b] + k//CPT
        dstrel_u = np.full((128, cfg.NWIN * cfg.TPW), -1.0, np.float16)

        call = 0
        for b in range(cfg.NBUCK):
            nb = cfg.ncwb[b]
            sel = eb == b
            wb, kb, pb = ew[sel], k[sel], p[sel]
            srcfull = np.zeros((cfg.NWIN, nb, WIN), np.int16)
            srcfull[wb, kb, pb] = (es[sel] - b * cfg.BUCKSZ).astype(np.int16)
            segfull = np.full((cfg.NWIN, nb, WIN), -1.0, np.float16)
            segfull[wb, kb, pb] = segloc[sel].astype(np.float16)
            seg_u[:, cfg.segoff[b]:cfg.segoff[b + 1]] = \
                segfull.transpose(2, 0, 1).reshape(128, -1)
            dstrel_u[32 * (kb % CPT) + segloc[sel],
                     wb * cfg.TPW + cfg.tbase[b] + kb // CPT] = \
                dl[sel].astype(np.float16)
            for (w0, wc) in cfg.WGROUPS:
                arr = srcfull[w0:w0 + wc].reshape(-1)
                ncol = len(arr) // 16
                idx_cols[:, cfg.idxoff[call]:cfg.idxoff[call] + ncol] = \
                    arr.reshape(-1, 16).T
                call += 1
        srcidx_u = np.tile(idx_cols, (8, 1)).copy()

        shard = slice(c * cfg.SHARD, (c + 1) * cfg.SHARD)
        xs_pad = np.zeros((cfg.SHARD_PAD, cfg.F), np.float16)
        xs_pad[: cfg.SHARD] = xp[shard]
        xs_u = xs_pad.reshape(cfg.NWIN, WIN, cfg.F) \
            .transpose(1, 0, 2).reshape(128, -1).copy()
        dinv_pad = np.zeros(cfg.SHARD_PAD, np.float32)
        dinv_pad[: cfg.SHARD] = dinv[shard]
        dinv_u = dinv_pad.reshape(cfg.NWIN, WIN).T.copy()

        im = {"xfull": xfull, "xs": xs_u, "dinvw": dinv_u,
              "srcidx": srcidx_u, "seg": seg_u, "dstrel": dstrel_u}
        im.update(consts)
        in_maps.append(im)
    return in_maps


def _emit_gather(nc, out_ap, in_ap, idxs_ap, num_idxs, elem_size, elem_step):
    """dma_gather with sub-256B element reads (row stride still 256B-aligned)."""
    gp = nc.gpsimd
    stride_bytes = elem_step * mybir.dt.size(in_ap.dtype)
    assert stride_bytes % 256 == 0
    assert ap_utils.ap_is_contiguous(out_ap.ap[1:])
    assert ap_utils.ap_is_contiguous(idxs_ap.ap[1:])
    _in_ap = gp.lower_ap_dma(in_ap, for_custom_bir_dma=True)
    _idxs_ap = gp.lower_ap(idxs_ap)
    _out_ap = gp.lower_ap(out_ap)
    return gp.add_instruction(
        mybir.InstDMAGatherAnt(
            name=nc.get_next_instruction_name(),
            ins=[*_in_ap, _idxs_ap, gp.lower_val_access(gp.to_reg(num_idxs))],
            outs=[_out_ap],
            transpose=False, num_idxs=num_idxs, elem_size=elem_size,
            stride_bytes_256=stride_bytes // 256, gen_mode=0,
            single_packet=False, queue_num=0, sbuf_tokens_per_rank=0,
            sbuf_free_dim_per_rank=0, sbuf_free_dim_pad_per_rank=0,
            sbuf_byte_offset=0,
        )
    )


def build_program(cfg):
    F, HID, NCLS = cfg.F, cfg.HID, cfg.NCLS
    nc = bacc.Bacc("TRN2", target_bir_lowering=False, debug=False,
                   num_devices=N_CORES)

    dt_in = {
        "xfull": ([cfg.NPAD, 2 * F], F16),
        "xs": ([128, cfg.NWIN * F], F16),
        "dinvw": ([128, cfg.NWIN], F32),
        "srcidx": ([128, cfg.IDXCOLS], I16),
        "seg": ([128, cfg.segoff[-1]], F16),
        "dstrel": ([128, cfg.NWIN * cfg.TPW], F16),
        "iota32": ([128, NSLOT], F16), "iota128": ([128, WIN], F16),
        "eye128": ([WIN, WIN], F16), "eyeh": ([HID, HID], F16),
        "eyec": ([NCLS, NCLS], F16),
        "w1": ([F, HID], F16), "w2": ([HID, NCLS], F16),
        "b1t": ([128, HID], F32), "b2t": ([128, NCLS], F32),
    }
    d = {kk: nc.dram_tensor(kk, sh, dt, kind="ExternalInput")
         for kk, (sh, dt) in dt_in.items()}
    out = nc.dram_tensor("out", [cfg.SHARD, NCLS], F32, kind="ExternalOutput")
    h1cin = nc.dram_tensor("h1cin", [cfg.SHARD_PAD, 2 * HID], F16)
    h1cout = nc.dram_tensor("h1cout", [cfg.NPAD, 2 * HID], F16,
                            addr_space="Shared")

    with tile.TileContext(nc) as tc:
        with tc.tile_pool(name="res", bufs=1) as res, \
             tc.tile_pool(name="stg", bufs=3) as stg, \
             tc.tile_pool(name="sbld", bufs=3) as sbld, \
             tc.tile_pool(name="epi", bufs=3) as epi, \
             tc.tile_pool(name="ps1", bufs=2, space="PSUM") as ps1, \
             tc.tile_pool(name="ps2", bufs=2, space="PSUM") as ps2, \
             tc.tile_pool(name="ps3", bufs=2, space="PSUM") as ps3, \
             tc.tile_pool(name="ps4", bufs=2, space="PSUM") as ps4:

            r = {}
            for kk in ["xs", "dinvw", "srcidx", "seg", "dstrel", "iota32",
                       "iota128", "eye128", "eyeh", "eyec", "w1", "w2",
                       "b1t", "b2t"]:
                sh, dt = dt_in[kk]
                r[kk] = res.tile(sh, dt, tag=kk, name=kk)
                nc.sync.dma_start(r[kk][:], d[kk][:])
            partials = res.tile([128, cfg.NWIN * cfg.TPW * F], F16, tag="pa")
            pview = partials[:].rearrange("p (w x) -> p w x", w=cfg.NWIN)
            h1sb = res.tile([128, cfg.NWIN, 2 * HID], F16, tag="h1sb")
            nc.gpsimd.memset(h1sb[:], 0.0)
            zero32 = res.tile([128, NSLOT], F16, tag="zero32")
            nc.gpsimd.memset(zero32[:], 0.0)
            xs_v = r["xs"][:].rearrange("p (w f) -> p w f", w=cfg.NWIN)

            sub = int(__import__("os").environ.get("K_SUB", "9"))

            def edge_phase(xfull_dram, din):
                call = 0
                for b in range(cfg.NBUCK):
                    nb = cfg.ncwb[b]
                    nptb = cfg.npt[b]
                    jw = max(1, min(4, 512 // max(1, nptb * F)))
                    blo = b * cfg.BUCKSZ
                    bhi = min(cfg.NPAD, blo + cfg.BUCKSZ)
                    src_view = xfull_dram[blo:bhi, 0:din]
                    for (w0, wc) in cfg.WGROUPS:
                        nidx = wc * nb * WIN
                        stage = stg.tile([128, WG * nb * F], F16, tag="stage")
                        stg_v = stage[:].rearrange("p (c f) -> p c f", f=F)
                        idx_ap = r["srcidx"][:, cfg.idxoff[call]:
                                             cfg.idxoff[call] + nidx // 16]
                        if sub >= 2:
                            _emit_gather(nc, stg_v[:, 0:wc * nb, :], src_view,
                                         idx_ap, nidx, din, 2 * F)
                        else:
                            nc.gpsimd.memset(stage[:], 0.0)
                        call += 1
                        if sub < 3:
                            continue
                        s32 = sbld.tile([128, WG * nb, NSLOT], F16, tag="s32")
                        soff = cfg.segoff[b] + w0 * nb
                        seg_b = r["seg"][:, soff:soff + wc * nb] \
                            .unsqueeze(2).broadcast_to([128, wc * nb, NSLOT])
                        iota_b = r["iota32"][:].unsqueeze(1) \
                            .broadcast_to([128, wc * nb, NSLOT])
                        nc.vector.tensor_tensor(s32[:, 0:wc * nb, :], seg_b,
                                                iota_b, AluOpType.is_equal)
                        for j0 in range(0, wc, jw):
                            jn = min(jw, wc - j0)
                            pt = ps1.tile([128, jw, nptb * F], F32, tag="ps1")
                            for j in range(jn):
                                for k in range(nptb * CPT):
                                    sl = 32 * (k % CPT)
                                    virt = k >= nb
                                    ch = (j0 + j) * nb + (0 if virt else k)
                                    nc.tensor.matmul(
                                        pt[sl:sl + 32, j,
                                           (k // CPT) * F:(k // CPT) * F + F],
                                        zero32[:] if virt else s32[:, ch, :],
                                        stg_v[:, ch, :],
                                        start=True, stop=True,
                                        tile_position=(0, sl))
                            # zero any never-written tail slots of partial tiles
                            dst = pview[:, w0 + j0:w0 + j0 + jn,
                                        (cfg.tbase[b]) * F:
                                        (cfg.tbase[b] + nptb) * F]
                            nc.scalar.copy(dst, pt[:, 0:jn, :])

            def dense_phase(layer, din, dout, w_t, bias_t, eye_t):
                for (w0, wc) in cfg.L2GROUPS:
                    s2 = sbld.tile([128, L2B * cfg.TPW, WIN], F16, tag="s2")
                    doff = w0 * cfg.TPW
                    ncol = wc * cfg.TPW
                    dr_b = r["dstrel"][:, doff:doff + ncol] \
                        .unsqueeze(2).broadcast_to([128, ncol, WIN])
                    iota_b = r["iota128"][:].unsqueeze(1) \
                        .broadcast_to([128, ncol, WIN])
                    nc.vector.tensor_tensor(s2[:, 0:ncol, :], dr_b, iota_b,
                                            AluOpType.is_equal)
                    p2 = ps2.tile([din, L2B, WIN], F32, tag="ps2")
                    for j in range(wc):
                        w = w0 + j
                        for t in range(cfg.TPW):
                            nc.tensor.matmul(
                                p2[:, j, :],
                                pview[:, w, t * F:t * F + din],
                                s2[:, j * cfg.TPW + t, :],
                                start=(t == 0), stop=False)
                        sl_src = xs_v if layer == 1 else h1sb[:]
                        nc.tensor.matmul(p2[:, j, :], sl_src[:, w, 0:din],
                                         r["eye128"][:], start=False, stop=True)
                    zt = epi.tile([din, L2B, WIN], F16, tag="zt")
                    nc.scalar.copy(zt[:, 0:wc, :], p2[:, 0:wc, :])
                    p3 = ps3.tile([dout, L2B, WIN], F32, tag="ps3")
                    nc.tensor.matmul(p3[:, 0:wc, :], w_t[:], zt[:, 0:wc, :],
                                     start=True, stop=True)
                    ot = epi.tile([dout, L2B, WIN], F16, tag="ot")
                    nc.scalar.copy(ot[:, 0:wc, :], p3[:, 0:wc, :])
                    p4 = ps4.tile([128, L2B, dout], F32, tag="ps4")
                    for j in range(wc):
                        nc.tensor.matmul(p4[:, j, :], ot[:, j, :], eye_t[:],
                                         start=True, stop=True)
                    tmp = epi.tile([128, L2B, dout], F32, tag="tmp")
                    for j in range(wc):
                        w = w0 + j
                        nc.vector.scalar_tensor_tensor(
                            tmp[:, j, :], p4[:, j, :], r["dinvw"][:, w:w + 1],
                            bias_t[:, 0:dout], AluOpType.mult, AluOpType.add)
                    if layer == 1:
                        for j in range(wc):
                            w = w0 + j
                            nc.scalar.activation(
                                h1sb[:, w, 0:dout], tmp[:, j, :], AFT.Relu,
                                scale=r["dinvw"][:, w:w + 1])
                    else:
                        mxn = epi.tile([128, L2B], F32, tag="mxn")
                        nc.vector.reduce_max(mxn[:, 0:wc], tmp[:, 0:wc, :],
                                             axis=mybir.AxisListType.X,
                                             negate=True)
                        exps = epi.tile([128, L2B, dout], F32, tag="exps")
                        for j in range(wc):
                            nc.scalar.activation(exps[:, j, :], tmp[:, j, :],
                                                 AFT.Exp, bias=mxn[:, j:j + 1])
                        sums = epi.tile([128, L2B], F32, tag="sums")
                        nc.vector.reduce_sum(sums[:, 0:wc], exps[:, 0:wc, :],
                                             axis=mybir.AxisListType.X)
                        lns = epi.tile([128, L2B], F32, tag="lns")
                        nc.scalar.activation(lns[:, 0:wc], sums[:, 0:wc],
                                             AFT.Ln)
                        ob = epi.tile([128, L2B, dout], F32, tag="ob")
                        for j in range(wc):
                            nc.vector.tensor_scalar(
                                ob[:, j, :], tmp[:, j, :], mxn[:, j:j + 1],
                                lns[:, j:j + 1], AluOpType.add,
                                AluOpType.subtract)
                        for j in range(wc):
                            w = w0 + j
                            rlo = w * WIN
                            rhi = min(cfg.SHARD, rlo + WIN)
                            if rhi > rlo:
                                nc.sync.dma_start(out[rlo:rhi, :],
                                                  ob[0:rhi - rlo, j, :])

            import os as _os
            stages = int(_os.environ.get("K_STAGES", "5"))
            repeat = int(_os.environ.get("K_REPEAT", "1"))
            for _rep in range(repeat):
                if stages >= 1:
                    edge_phase(d["xfull"], F)
                if stages >= 2:
                    dense_phase(1, F, HID, r["w1"], r["b1t"], r["eyeh"])
                if stages >= 3:
                    nc.sync.dma_start(
                        h1cin.rearrange("(p w) f -> p w f", p=128), h1sb[:])
                    nc.gpsimd.collective_compute(
                        "AllGather", mybir.AluOpType.bypass,
                        ins=[h1cin.ap().opt()], outs=[h1cout.ap().opt()],
                        replica_groups=[list(range(N_CORES))])
                if stages >= 4:
                    edge_phase(h1cout, HID)
                if stages >= 5:
                    dense_phase(2, HID, NCLS, r["w2"], r["b2t"], r["eyec"])
            if stages < 5:
                # touch 'out' so the output tensor is produced
                dbg = epi.tile([128, NCLS], F32, tag="dbg", name="dbg")
                nc.gpsimd.memset(dbg[:], 0.0)
                nc.sync.dma_start(out[0:128, :], dbg[:])

    nc.compile()
    return nc


_CACHE = {}


def kernel(x, edge_index, W1, b1, W2, b2):
    x = np.asarray(x)
    cfg = Cfg(x.shape[0], np.asarray(edge_index).shape[1],
              f_in=x.shape[1], hid=np.asarray(W1).shape[1],
              ncls=np.asarray(W2).shape[1])
    in_maps = host_prep(cfg, x, edge_index, W1, b1, W2, b2)
    key = (cfg.N, cfg.E, cfg.F, cfg.HID, cfg.NCLS, tuple(cfg.ncwb))
    if key not in _CACHE:
        _CACHE[key] = build_program(cfg)
    nc = _CACHE[key]
    res = bass_utils.run_bass_kernel_spmd(nc, in_maps,
                                          core_ids=list(range(N_CORES)))
    return np.concatenate([res.results[c]["out"] for c in range(N_CORES)],
                          axis=0)
